# revision 1
# baseline (speedup 1.0000x reference)
"""GTE program-classification kernel for 8 Trainium2 NeuronCores.

Data-parallel over dst nodes: each core handles 1024 of the 8192 dst nodes.
Per-core: embedding row gather (indirect DMA) -> 2-layer post-norm
transformer over the 8-message mailbox -> max-pool -> linear classifier.

v2 redesign vs baseline:
- software-pipelined emission order: per-engine instruction streams are
  interleaved across tiles so in-order sequencers never head-of-line block
  (attention of tile i+2 is emitted before the FFN of tile i).
- whole-tile DmaTransposeAnt (1 instr per transpose set instead of 32)
- residual folded into PSUM via preload + start=False matmul accumulation
- LN stats ride the PSUM drain (ACT accum_out) + one Square pass; the
  rstd chain runs on Pool/ACT so the DVE stream stays pure attention
- attention tree reduce tails and maxpool offloaded to the Pool engine
- pn broadcast replaced by pair-duplicated pn2 + strided AV multiply (2x DVE)
"""
import sys
if '/opt/trn_rl_repo' not in sys.path:
    sys.path.insert(0, '/opt/trn_rl_repo')

import numpy as np
import ml_dtypes

import concourse.bass as bass
import concourse.tile as tile
import concourse.mybir as mybir
from concourse.bass import ds
from concourse.bass_utils import run_bass_kernel_spmd

F32 = mybir.dt.float32
BF16 = mybir.dt.bfloat16
I32 = mybir.dt.int32
AF = mybir.ActivationFunctionType
OP = mybir.AluOpType
AX = mybir.AxisListType

P = 128
D = 512
H = 8
DH = 64
S = 8          # messages used per node (9th dropped by the reference)
NL = 2
V = 50000
NCLS = 104
DFF = 1024
NDST = 8192
NSRC = 40000
NCORES = 8
NLOC = NDST // NCORES      # 1024 dst nodes per core
NT = NLOC // P             # 8 node tiles per core
DC = D // P                # 4 d-chunks
FCH = DFF // P             # 8 dff-chunks
LN_EPS = 1e-5

# instruction-name -> phase label, filled during build for profiling
PHASE_OF = {}


def _split_multiwait_drains(nc):
    """walrus in this container accepts only one sync-wait per instruction;
    split any multi-wait Drain into a chain of single-wait drains."""
    for fn in nc.m.functions:
        for bb in fn.blocks:
            newlist = []
            for ins in bb.instructions:
                si = ins.sync_info
                if si is not None and si.on_wait and len(si.on_wait) > 1:
                    waits = list(si.on_wait)
                    for j, w in enumerate(waits[:-1]):
                        d = mybir.InstDrain(name=f'{ins.name}-sw{j}',
                                            engine=ins.engine)
                        d.sync_info = mybir.SyncInfo(on_wait=[w], on_update=[])
                        newlist.append(d)
                    si.on_wait = [waits[-1]]
                newlist.append(ins)
            bb.instructions[:] = newlist


DEBUG_DUMPS = False


def build(flags):
    nc = bass.Bass()
    dbg = {}
    if DEBUG_DUMPS:
        dbg['x0'] = nc.dram_tensor("dbg_x0", [P, S, D], BF16,
                                   kind="ExternalOutput")
        dbg['qkv0'] = nc.dram_tensor("dbg_qkv0", [P, 3, S, D], BF16,
                                     kind="ExternalOutput")
        dbg['pexp0'] = nc.dram_tensor("dbg_pexp0", [P, S, H, S], F32,
                                      kind="ExternalOutput")
        dbg['a0'] = nc.dram_tensor("dbg_a0", [P, S, D], BF16,
                                   kind="ExternalOutput")
        dbg['stt0'] = nc.dram_tensor("dbg_stt0", [P, 4 * S], F32,
                                     kind="ExternalOutput")
        dbg['xln1'] = nc.dram_tensor("dbg_xln1", [P, S, D], BF16,
                                     kind="ExternalOutput")
        dbg['xl0'] = nc.dram_tensor("dbg_xl0", [P, S, D], BF16,
                                    kind="ExternalOutput")

    emb_d = nc.dram_tensor("embb", [V, D], BF16, kind="ExternalInput")
    idx_d = nc.dram_tensor("tid2", [NLOC, S], I32, kind="ExternalInput")
    # wqkvT has the q-block pre-scaled by 1/8 on the host
    wq_d = nc.dram_tensor("wqkvT", [NL, D, 3 * D], BF16, kind="ExternalInput")
    wo_d = nc.dram_tensor("woT", [NL, D, D], BF16, kind="ExternalInput")
    w1_d = nc.dram_tensor("w1T", [NL, D, DFF], BF16, kind="ExternalInput")
    w2_d = nc.dram_tensor("w2T", [NL, DFF, D], BF16, kind="ExternalInput")
    wf_d = nc.dram_tensor("wfcT", [D, NCLS], BF16, kind="ExternalInput")
    out_d = nc.dram_tensor("logits", [NLOC, NCLS], F32, kind="ExternalOutput")

    need_vec = {}
    if flags['bqkv']:
        need_vec['bqkv'] = [NL, 3 * D]
    if flags['bo']:
        need_vec['bo'] = [NL, D]
    if flags['b2']:
        need_vec['b2'] = [NL, D]
    if flags['bfc']:
        need_vec['bfc'] = [1, NCLS]
    if flags['ln_g']:
        need_vec['ln1_g'] = [NL, D]
        need_vec['ln2_g'] = [NL, D]
    if flags['ln_b']:
        need_vec['ln1_b'] = [NL, D]
        need_vec['ln2_b'] = [NL, D]
    vec_d = {k: nc.dram_tensor(k, shp, F32, kind="ExternalInput")
             for k, shp in need_vec.items()}
    b1t_d = (nc.dram_tensor("b1t", [P, NL * FCH], F32, kind="ExternalInput")
             if flags['b1'] else None)

    with tile.TileContext(nc) as tc:
        with tc.tile_pool(name="wp", bufs=1) as wp, \
             tc.tile_pool(name="tp", bufs=2) as tp, \
             tc.tile_pool(name="psA", bufs=2, space="PSUM") as psA, \
             tc.tile_pool(name="psB", bufs=2, space="PSUM") as psB:

            # ---- resident weights (bf16), one DMA each ----
            wq_sb, wo_sb, w1_sb, w2_sb = [], [], [], []
            for l in range(NL):
                t = wp.tile([P, DC, 3 * D], BF16, tag=f"wq{l}", name=f"wq{l}")
                nc.sync.dma_start(
                    t[:], wq_d[l].rearrange("(c p) n -> p c n", p=P))
                wq_sb.append(t)
                t = wp.tile([P, DC, D], BF16, tag=f"wo{l}", name=f"wo{l}")
                nc.sync.dma_start(
                    t[:], wo_d[l].rearrange("(c p) n -> p c n", p=P))
                wo_sb.append(t)
                t = wp.tile([P, DC, DFF], BF16, tag=f"w1{l}", name=f"w1{l}")
                nc.sync.dma_start(
                    t[:], w1_d[l].rearrange("(c p) n -> p c n", p=P))
                w1_sb.append(t)
                t = wp.tile([P, FCH, D], BF16, tag=f"w2{l}", name=f"w2{l}")
                nc.sync.dma_start(
                    t[:], w2_d[l].rearrange("(c p) n -> p c n", p=P))
                w2_sb.append(t)
            wf_sb = wp.tile([P, DC, NCLS], BF16, tag="wf", name="wf")
            nc.sync.dma_start(wf_sb[:],
                              wf_d[:].rearrange("(c p) n -> p c n", p=P))

            vec_sb = {}
            for k, shp in need_vec.items():
                n = shp[0] * shp[1]
                t0 = wp.tile([1, n], F32, tag=f"{k}_row", name=f"{k}_row")
                nc.sync.dma_start(t0[:, :],
                                  vec_d[k][:].rearrange("a b -> 1 (a b)"))
                tb = wp.tile([P, n], F32, tag=f"{k}_rep", name=f"{k}_rep")
                nc.gpsimd.partition_broadcast(tb[:], t0[:])
                vec_sb[k] = tb

            b1t_sb = None
            if flags['b1']:
                b1t_sb = wp.tile([P, NL * FCH], F32, tag="b1t", name="b1t")
                nc.sync.dma_start(b1t_sb[:], b1t_d[:])

            def vsl(k, l, n):
                return vec_sb[k][:, l * n:(l + 1) * n]

            eps_sb = wp.tile([P, 1], F32, tag="eps", name="eps")
            nc.vector.memset(eps_sb[:], LN_EPS)

            st8 = [None] * NT      # per-tile state

            def _mark(label, fn_, *args):
                before = {ins.name
                          for f in nc.m.functions
                          for bb in f.blocks
                          for ins in bb.instructions}
                fn_(*args)
                for f in nc.m.functions:
                    for bb in f.blocks:
                        for ins in bb.instructions:
                            if ins.name not in before:
                                PHASE_OF[ins.name] = label

            # ---------------- phase G: gather ----------------
            def phG(i):
                st = {'i': i}
                st8[i] = st
                idx_sb = tp.tile([P, S], I32, tag="idx", name="idx")
                nc.sync.dma_start(idx_sb[:], idx_d[ds(i * P, P), :])
                x = tp.tile([P, S, D], BF16, tag="x", bufs=3, name="x")
                st['x'] = x
                for s in range(S):
                    nc.gpsimd.indirect_dma_start(
                        out=x[:, s, :], out_offset=None, in_=emb_d[:],
                        in_offset=bass.IndirectOffsetOnAxis(
                            ap=idx_sb[:, s:s + 1], axis=0))
                if DEBUG_DUMPS and i == 0:
                    nc.sync.dma_start(dbg['x0'][:], x[:])

            # ---------------- phase F: (transpose +) QKV ----------------
            def phF(i, l):
                st = st8[i]
                if l == 0:
                    x = st['x']
                    xT = tp.tile([P, S, DC, P], BF16, tag="T", bufs=3,
                                 name="xT")
                    nc.sync.dma_start_transpose(xT[:], x[:])
                    xsum = tp.tile([P, S], F32, tag="xsum", name="xsum")
                    st['xsum'] = xsum
                    for s in range(S):
                        nc.vector.reduce_sum(xsum[:, s:s + 1], x[:, s, :],
                                             axis=AX.X)
                else:
                    xT = st['xTn']   # built slice-wise by phB(i,0)'s LN2
                qkv = tp.tile([P, 3, S, D], BF16, tag="qkv", name="qkv")
                st['qkv'] = qkv
                for s in range(S):
                    pq = psA.tile([P, 3 * D], F32, tag="pq", name="pq")
                    for c in range(DC):
                        lhsT = xT[:, s, c, :]
                        for nb in range(3):
                            nc.tensor.matmul(
                                pq[:, nb * D:(nb + 1) * D], lhsT,
                                wq_sb[l][:, c, nb * D:(nb + 1) * D],
                                start=(c == 0), stop=(c == DC - 1))
                    if flags['bqkv']:
                        nc.vector.tensor_add(pq[:], pq[:],
                                             vsl('bqkv', l, 3 * D))
                    nc.scalar.copy(qkv[:, :, s, :], pq[:])
                if DEBUG_DUMPS and i == 0 and l == 0:
                    nc.sync.dma_start(dbg['qkv0'][:], qkv[:])

            # -------- phase A part 1: scores + exp --------
            POOL_QK_S = frozenset()    # s-slices whose qk tree runs on Pool

            def phA_sc(i, l):
                st = st8[i]
                qkv = st['qkv']
                scores = tp.tile([P, S, H, S], F32, tag="scores", bufs=1,
                                 name="scores")
                st['scores'] = scores
                for s in range(S):
                    qk = tp.tile([P, S, D], BF16, tag="qkav", bufs=3, name="qk")
                    nc.vector.tensor_tensor(
                        out=qk[:],
                        in0=qkv[:, 1, :, :],
                        in1=qkv[:, 0, s, :].unsqueeze(1)
                            .broadcast_to([P, S, D]),
                        op=OP.mult)
                    qk4 = qk[:].rearrange("p t (h e) -> p t h e", h=H)
                    nc.vector.tensor_add(qk4[:, :, :, 0:32],
                                         qk4[:, :, :, 0:32],
                                         qk4[:, :, :, 32:64])
                    nc.vector.tensor_add(qk4[:, :, :, 0:16],
                                         qk4[:, :, :, 0:16],
                                         qk4[:, :, :, 16:32])
                    nc.vector.tensor_add(qk4[:, :, :, 0:8],
                                         qk4[:, :, :, 0:8],
                                         qk4[:, :, :, 8:16])
                    nc.vector.reduce_sum(
                        scores[:, s, :, :].transpose([0, 2, 1]),
                        qk4[:, :, :, 0:8], axis=AX.X)
                    # exp per half so AV can start before all s are scored
                    if s % 4 == 3:
                        hs = s - 3
                        nc.scalar.activation(
                            scores[:, hs:s + 1, :, :]
                            .rearrange("p s h t -> p (s h t)"),
                            scores[:, hs:s + 1, :, :]
                            .rearrange("p s h t -> p (s h t)"), AF.Exp)

            # -------- phase A part 2: softmax tail + AV + aT --------
            def phA_av(i, l):
                st = st8[i]
                qkv = st['qkv']
                scores = st['scores']
                den = tp.tile([P, S * H], F32, tag="den", bufs=1, name="den")
                pn = tp.tile([P, S, H, S], BF16, tag="pn", bufs=1, name="pn")
                # pn2 is t-major so (t,h) merge to one AP dim in the AV mult
                pn2 = tp.tile([P, S, S, H, 2], BF16, tag="pn2", bufs=1,
                              name="pn2")
                denv = den[:].rearrange("p (s h) -> p s h", s=S)
                for hs in (0, 4):
                    sl = slice(hs, hs + 4)
                    nc.vector.reduce_sum(denv[:, sl, :], scores[:, sl, :, :],
                                         axis=AX.X)
                    nc.vector.reciprocal(den[:, hs * H:(hs + 4) * H],
                                         den[:, hs * H:(hs + 4) * H])
                    nc.vector.tensor_tensor(
                        out=pn[:, sl, :, :], in0=scores[:, sl, :, :],
                        in1=denv[:, sl, :].unsqueeze(3)
                            .broadcast_to([P, 4, H, S]),
                        op=OP.mult)
                    for s in range(hs, hs + 4):
                        nc.scalar.copy(
                            pn2[:, s, :, :, :],
                            pn[:, s, :, :].transpose([0, 2, 1]).unsqueeze(3)
                            .broadcast_to([P, S, H, 2]))

                # AV; result written into the (dead) q slot of qkv
                aT = tp.tile([P, S, DC, P], BF16, tag="T", bufs=3, name="aT")
                st['aT'] = aT
                if DEBUG_DUMPS and i == 0 and l == 0:
                    nc.sync.dma_start(dbg['pexp0'][:], scores[:])
                for s in range(S):
                    av = tp.tile([P, S, D], BF16, tag="qkav", bufs=3, name="av")
                    av4 = av[:].rearrange(
                        "p t (h e) -> p t h e", h=H).rearrange(
                        "p t h (e2 two) -> p (t h) e2 two", two=2)
                    v4 = qkv[:, 2, :, :].rearrange(
                        "p t (h e) -> p t h e", h=H).rearrange(
                        "p t h (e2 two) -> p (t h) e2 two", two=2)
                    pnx = pn2[:, s, :, :, :].rearrange(
                        "p t h two -> p (t h) two") \
                        .unsqueeze(2).broadcast_to([P, S * H, 32, 2])
                    nc.vector.tensor_tensor(out=av4, in0=v4, in1=pnx,
                                            op=OP.mult)
                    avf = av[:]
                    nc.vector.tensor_add(avf[:, 0:4, :], avf[:, 0:4, :],
                                         avf[:, 4:8, :])
                    nc.vector.tensor_add(avf[:, 0:2, :], avf[:, 0:2, :],
                                         avf[:, 2:4, :])
                    nc.gpsimd.tensor_tensor(
                        out=qkv[:, 0, s, :], in0=avf[:, 0, :],
                        in1=avf[:, 1, :], op=OP.add)
                    nc.sync.dma_start_transpose(aT[:, s, :, :],
                                                qkv[:, 0, s, :])
                if DEBUG_DUMPS and i == 0 and l == 0:
                    nc.sync.dma_start(dbg['a0'][:], qkv[:, 0, :, :])

            # ---- shared: accumulate-drain + LN finish + normalize ----
            def ln_drain(ps_t, x, stt, s, scr):
                """drain psum->scr with sum accum, residual-add into x[s]
                (Pool), then sumsq via ACT Square+accum. sum(x+out) =
                sum(out) when x is post-LN (mean exactly 0); for the raw
                layer-0 input the caller passes xsum to add sum(x)."""
                nc.scalar.activation(scr[:], ps_t[:], AF.Identity,
                                     accum_out=stt[:, s:s + 1])
                nc.gpsimd.tensor_add(x[:, s, :], x[:, s, :], scr[:])
                nc.scalar.activation(scr[:], x[:, s, :], AF.Square,
                                     accum_out=stt[:, S + s:S + s + 1])

            def ln_finish(stt, x, s0, nh, gk, bk, l, xT_out):
                """finish LN stats for s in [s0, s0+nh) and normalize; when
                xT_out is given, transpose each normalized slice into it."""
                sl = slice(s0, s0 + nh)
                msum = stt[:, 0:S][:, sl]
                qsum = stt[:, S:2 * S][:, sl]
                var = stt[:, 2 * S:3 * S][:, sl]
                rstd = var
                tmp = stt[:, 3 * S:4 * S][:, sl]
                nmr = tmp                # tmp is dead once var is formed
                nc.vector.scalar_tensor_tensor(
                    out=tmp[:], in0=msum[:], scalar=1.0 / (D * D),
                    in1=msum[:], op0=OP.mult, op1=OP.mult)
                nc.vector.scalar_tensor_tensor(
                    out=var[:], in0=qsum[:], scalar=1.0 / D,
                    in1=tmp[:], op0=OP.mult, op1=OP.subtract)
                # rstd = (var+eps)^-0.5 = exp(-0.5*ln(var+eps)); Rsqrt is
                # blocked in this bass for accuracy, and this stays off DVE
                nc.scalar.activation(var[:], var[:], AF.Ln,
                                     bias=eps_sb[:, 0:1])
                nc.scalar.activation(rstd[:], var[:], AF.Exp, scale=-0.5)
                nc.vector.scalar_tensor_tensor(
                    out=nmr[:], in0=msum[:], scalar=-1.0 / D,
                    in1=rstd[:], op0=OP.mult, op1=OP.mult)
                for j in range(nh):
                    s = s0 + j
                    nc.scalar.activation(x[:, s, :], x[:, s, :],
                                         AF.Identity,
                                         bias=nmr[:, j:j + 1],
                                         scale=rstd[:, j:j + 1])
                    if gk is not None:
                        nc.vector.tensor_tensor(out=x[:, s, :],
                                                in0=x[:, s, :],
                                                in1=vsl(gk, l, D), op=OP.mult)
                    if bk is not None:
                        nc.vector.tensor_tensor(out=x[:, s, :],
                                                in0=x[:, s, :],
                                                in1=vsl(bk, l, D), op=OP.add)
                    if xT_out is not None:
                        nc.sync.dma_start_transpose(xT_out[:, s, :, :],
                                                    x[:, s, :])

            # ---------------- phase B: Wo+LN1+FFN+LN2 (+tail) -------------
            def phB(i, l):
                st = st8[i]
                x = st['x']
                aT = st['aT']
                gk1 = 'ln1_g' if flags['ln_g'] else None
                bk1 = 'ln1_b' if flags['ln_b'] else None
                gk2 = 'ln2_g' if flags['ln_g'] else None
                bk2 = 'ln2_b' if flags['ln_b'] else None

                stt = tp.tile([P, 4 * S], F32, tag="lnstat", bufs=2,
                              name="stt")
                x1T = tp.tile([P, S, DC, P], BF16, tag="T", bufs=3,
                              name="x1T")
                dmp = DEBUG_DUMPS and i == 0 and l == 0
                for s in range(S):
                    po = psB.tile([P, D], F32, tag="mm", name="po")
                    for c in range(DC):
                        nc.tensor.matmul(po[:], aT[:, s, c, :],
                                         wo_sb[l][:, c, :],
                                         start=(c == 0), stop=(c == DC - 1))
                    if flags['bo']:
                        nc.vector.tensor_add(po[:], po[:], vsl('bo', l, D))
                    ln_drain(po, x, stt, s, st['qkv'][:, 0, s, :])
                    if s % 4 == 3:
                        if l == 0:
                            nc.vector.tensor_add(
                                stt[:, s - 3:s + 1], stt[:, s - 3:s + 1],
                                st['xsum'][:, s - 3:s + 1])
                        ln_finish(stt, x, s - 3, 4, gk1, bk1, l, x1T)

                if dmp:
                    nc.sync.dma_start(dbg['stt0'][:], stt[:])
                    nc.sync.dma_start(dbg['xln1'][:], x[:])
                stt2 = tp.tile([P, 4 * S], F32, tag="lnstat", bufs=2,
                               name="stt2")
                xTn = None
                if l == 0:
                    xTn = tp.tile([P, S, DC, P], BF16, tag="T", bufs=3,
                                  name="xTn")
                    st['xTn'] = xTn
                for hf in range(2):
                    hT = tp.tile([P, FCH, D], BF16, tag="hT", bufs=1,
                                 name="hT")
                    for m in range(FCH):
                        ph = psB.tile([P, D], F32, tag="mm", name="ph")
                        for c in range(DC):
                            nc.tensor.matmul(
                                ph[:],
                                w1_sb[l][:, c, m * P:(m + 1) * P],
                                x1T[:, hf * 4:(hf + 1) * 4, c, :],
                                start=(c == 0), stop=(c == DC - 1))
                        if flags['b1']:
                            nc.scalar.activation(
                                hT[:, m, :], ph[:], AF.Relu,
                                bias=b1t_sb[:, l * FCH + m:l * FCH + m + 1])
                        else:
                            nc.scalar.activation(hT[:, m, :], ph[:], AF.Relu)
                    for sh in range(4):
                        s = hf * 4 + sh
                        pf = psB.tile([P, D], F32, tag="mm", name="pf")
                        for k in range(FCH):
                            nc.tensor.matmul(
                                pf[:], hT[:, k, sh * P:(sh + 1) * P],
                                w2_sb[l][:, k, :],
                                start=(k == 0), stop=(k == FCH - 1))
                        if flags['b2']:
                            nc.vector.tensor_add(pf[:], pf[:],
                                                 vsl('b2', l, D))
                        ln_drain(pf, x, stt2, s, st['qkv'][:, 0, s, :])
                    ln_finish(stt2, x, hf * 4, 4, gk2, bk2, l, xTn)

                if dmp:
                    nc.sync.dma_start(dbg['xl0'][:], x[:])
                if l == NL - 1:
                    # max-pool over s (Pool engine) + classifier
                    nc.vector.tensor_tensor(out=x[:, 0:4, :],
                                            in0=x[:, 0:4, :],
                                            in1=x[:, 4:8, :], op=OP.max)
                    nc.vector.tensor_tensor(out=x[:, 0:2, :],
                                            in0=x[:, 0:2, :],
                                            in1=x[:, 2:4, :], op=OP.max)
                    nc.vector.tensor_tensor(out=x[:, 0, :], in0=x[:, 0, :],
                                            in1=x[:, 1, :], op=OP.max)
                    rT = tp.tile([P, DC, P], BF16, tag="rT", bufs=1,
                                 name="rT")
                    nc.sync.dma_start_transpose(rT[:], x[:, 0, :])
                    pc = psB.tile([P, D], F32, tag="mm", name="pc")
                    for c in range(DC):
                        nc.tensor.matmul(pc[:, 0:NCLS], rT[:, c, :],
                                         wf_sb[:, c, :],
                                         start=(c == 0), stop=(c == DC - 1))
                    if flags['bfc']:
                        nc.vector.tensor_add(pc[:, 0:NCLS], pc[:, 0:NCLS],
                                             vec_sb['bfc'][:, :])
                    lg = tp.tile([P, NCLS], F32, tag="lg", bufs=1,
                                 name="lg")
                    nc.vector.tensor_copy(lg[:], pc[:, 0:NCLS])
                    nc.sync.dma_start(out_d[ds(i * P, P), :], lg[:])

            # ---------- software-pipelined emission schedule ----------
            # DVE stream: Asc/av(0,0) (1,0) | (0,1) (2,0) | (1,1) (3,0) ...
            def G(i):
                _mark(f"G({i})", phG, i)

            def F(i, l):
                _mark(f"F({i},{l})", phF, i, l)

            def Asc(i, l):
                _mark(f"Asc({i},{l})", phA_sc, i, l)

            def Aav(i, l):
                _mark(f"Aav({i},{l})", phA_av, i, l)

            def B(i, l):
                _mark(f"B({i},{l})", phB, i, l)

            G(0); F(0, 0)
            G(1); F(1, 0)
            G(2)
            Asc(0, 0); Aav(0, 0)
            B(0, 0); F(0, 1)
            Asc(1, 0); Aav(1, 0)
            for i in range(NT):
                if i + 2 < NT:
                    F(i + 2, 0)
                if i + 1 < NT:
                    B(i + 1, 0)
                Asc(i, 1)
                if i + 1 < NT:
                    F(i + 1, 1)
                Aav(i, 1)
                B(i, 1)
                if i + 2 < NT:
                    Asc(i + 2, 0)
                    Aav(i + 2, 0)
                if i + 3 < NT:
                    G(i + 3)

    _split_multiwait_drains(nc)
    return nc


OPT_KEYS = ('bqkv', 'bo', 'b1', 'b2', 'bfc', 'ln_g', 'ln_b')
_cache = {}


def _get_nc(flags):
    key = tuple(flags[k] for k in OPT_KEYS)
    if key not in _cache:
        _cache[key] = build(flags)
    return _cache[key]


def _prep_common(inputs, flags):
    bf = ml_dtypes.bfloat16
    emb = np.asarray(inputs['emb'], dtype=np.float32)
    Wqkv = np.asarray(inputs['Wqkv'], dtype=np.float32)
    Wo = np.asarray(inputs['Wo'], dtype=np.float32)
    W1 = np.asarray(inputs['W1'], dtype=np.float32)
    W2 = np.asarray(inputs['W2'], dtype=np.float32)
    Wfc = np.asarray(inputs['Wfc'], dtype=np.float32)

    wqkvT = np.ascontiguousarray(Wqkv.transpose(0, 2, 1))
    wqkvT[:, :, 0:D] *= 0.125          # fold the 1/sqrt(dh) q-scale
    common = {
        'embb': emb.astype(bf),
        'wqkvT': wqkvT.astype(bf),
        'woT': np.ascontiguousarray(Wo.transpose(0, 2, 1)).astype(bf),
        'w1T': np.ascontiguousarray(W1.transpose(0, 2, 1)).astype(bf),
        'w2T': np.ascontiguousarray(W2.transpose(0, 2, 1)).astype(bf),
        'wfcT': np.ascontiguousarray(Wfc.T).astype(bf),
    }
    if flags['bqkv']:
        common['bqkv'] = np.asarray(inputs['bqkv'], dtype=np.float32)
    if flags['bo']:
        common['bo'] = np.asarray(inputs['bo'], dtype=np.float32)
    if flags['b1']:
        b1 = np.asarray(inputs['b1'], dtype=np.float32)
        common['b1t'] = np.ascontiguousarray(
            b1.reshape(NL, FCH, P).transpose(2, 0, 1).reshape(P, NL * FCH))
    if flags['b2']:
        common['b2'] = np.asarray(inputs['b2'], dtype=np.float32)
    if flags['bfc']:
        common['bfc'] = np.asarray(inputs['bfc'],
                                   dtype=np.float32).reshape(1, NCLS)
    if flags['ln_g']:
        common['ln1_g'] = np.asarray(inputs['ln1_g'], dtype=np.float32)
        common['ln2_g'] = np.asarray(inputs['ln2_g'], dtype=np.float32)
    if flags['ln_b']:
        common['ln1_b'] = np.asarray(inputs['ln1_b'], dtype=np.float32)
        common['ln2_b'] = np.asarray(inputs['ln2_b'], dtype=np.float32)
    return common


def _get_flags(inputs):
    return {
        'bqkv': bool(np.any(inputs['bqkv'])),
        'bo': bool(np.any(inputs['bo'])),
        'b1': bool(np.any(inputs['b1'])),
        'b2': bool(np.any(inputs['b2'])),
        'bfc': bool(np.any(inputs['bfc'])),
        'ln_g': bool(np.any(np.asarray(inputs['ln1_g']) != 1.0)
                     or np.any(np.asarray(inputs['ln2_g']) != 1.0)),
        'ln_b': bool(np.any(inputs['ln1_b']) or np.any(inputs['ln2_b'])),
    }


def kernel(**inputs):
    token_ids = np.asarray(inputs['token_ids'])
    edge_src = np.asarray(inputs['edge_src'])
    flags = _get_flags(inputs)
    nc = _get_nc(flags)
    common = _prep_common(inputs, flags)

    tid2 = token_ids[edge_src[:, :S]].astype(np.int32)     # [NDST, S]
    in_maps = []
    for c in range(NCORES):
        m = dict(common)
        m['tid2'] = np.ascontiguousarray(tid2[c * NLOC:(c + 1) * NLOC])
        in_maps.append(m)

    res = run_bass_kernel_spmd(nc, in_maps, core_ids=list(range(NCORES)))
    out = np.concatenate([res.results[c]['logits'] for c in range(NCORES)],
                         axis=0)
    return out.astype(np.float32)


if __name__ == '__main__':
    import time
    sys.path.insert(0, '/root/problem')
    import reference
    inp = {k: np.asarray(v) for k, v in reference.setup_inputs().items()}
    t0 = time.time()
    got = kernel(**inp)
    print(f"kernel ran in {time.time()-t0:.1f}s")
    exp = np.asarray(reference.reference(**reference.setup_inputs()))
    err = np.abs(got - exp).max()
    rel = err / np.abs(exp).max()
    print(f"absmax err {err:.3e}  rel {rel:.3e}")

